# revision 4
# baseline (speedup 1.0000x reference)
"""BlockDiagonalLowRankLinear Trainium2 kernel (v2: bf16 streams, big DMAs).

y = BlockDiag(blocks) @ x + U @ (V.T @ x), scaled by alpha, plus bias.

Shapes (full problem):
  x      [4, 2048, 4096] f32   -> flattened to [8192, 4096]
  blocks [16, 256, 256]  f32   (per-block [out, in])
  U      [4096, 64] f32, V [4096, 64] f32, bias [4096] f32, alpha [1] f32
  out    [4, 2048, 4096] f32

Sharding: data-parallel over tokens. Each of the 8 cores gets 1024 tokens
and the full (replicated) parameters; outputs are concatenated. No
collectives. Per-core HBM traffic is the floor: 16.8 MB x in + 16.8 MB
out per pass at ~358 GB/s => ~94 us; everything else hides under it.

v2 changes vs v1:
  - x loaded via SWDGE (gpsimd) cast-DMA f32->bf16, one 2 MB DMA per
    128-token t-chunk (was 4x512KB sync DMAs). bf16 halves PE transpose
    cost (1 cyc/row vs 2) and xT SBUF footprint. Tolerance is 2e-2;
    bf16 keeps rel err ~1e-3.
  - all matmul streams bf16 (PSUM accumulation stays f32).
  - output staged per t-chunk into a [128, 4096] f32 SBUF tile, stored
    with ONE 2 MB sync DMA (was 8x256KB). Loads ride the SWDGE ring,
    stores the SP HWDGE ring, so they interleave at the SDMA engines.
  - bias folded into the low-rank matmul as a 65th contraction row
    (lhsT row 64 = ones, rhs row 64 = bias), killing the per-chunk DVE
    bias-add and the [128, 4096] bias broadcast.
  - x prefetch distance 2 slabs (bufs=5) so phase-A transposes never
    stall on the same-iteration DMA.

Per-core algorithm (T=1024 tokens, D=4096, R=64, NB=16, bi=bo=256):
  Setup: alpha broadcast via tiny fp32 matmul (transient PSUM pool);
  blocks/U/V cast-loaded bf16 via SWDGE; blocks and U PE-transposed
  (bf16) and alpha-scaled into blocksT / ubT; bias cast-DMA'd into ubT
  row 64.
  Steady state, 4 slabs of 256 tokens, software-pipelined per o-chunk:
    B_bd(s,oc): per t-chunk PSUM acc [128,512]: 4 block-diag matmuls.
    A(s+1,oc): 8 PE transposes of next slab's x -> bf16 psum, ACT copy
      to xT, 4 V^T x matmuls accumulating tlr.
    B_fin(s,oc-1): U-term+bias matmul (K=65) closes acc, DVE copies acc
      into the o_sb row tile; after oc=7 one 2 MB DMA per t-chunk.
"""

import numpy as np

import concourse.bacc as bacc
import concourse.bass as bass
import concourse.mybir as mybir
import concourse.tile as tile
from concourse.bass_utils import run_bass_kernel_spmd
from concourse.masks import make_identity

F32 = mybir.dt.float32
BF16 = mybir.dt.bfloat16

N_CORES = 8
D = 4096          # in = out features
R = 64            # low rank
NB = 16           # diagonal blocks
BI = 256          # block in/out size
NK = D // 128     # 32 i-chunks
T_CORE = 1024     # tokens per core
T_SLAB = 256      # tokens per slab
OC = 512          # output column chunk
XBUFS = 5         # x staging ring: 2 tiles/slab, prefetch distance 2


def build(t_core: int = T_CORE, repeats: int = 1):
    nc = bacc.Bacc("TRN2", target_bir_lowering=False, debug=False)
    x = nc.declare_dram_parameter("x", [t_core, D], F32, isOutput=False)
    blocks = nc.declare_dram_parameter("blocks", [NB, BI, BI], F32, isOutput=False)
    U = nc.declare_dram_parameter("U", [D, R], F32, isOutput=False)
    V = nc.declare_dram_parameter("V", [D, R], F32, isOutput=False)
    bias = nc.declare_dram_parameter("bias", [D], F32, isOutput=False)
    alpha = nc.declare_dram_parameter("alpha", [1], F32, isOutput=False)
    out = nc.declare_dram_parameter("out", [t_core, D], F32, isOutput=True)

    n_slab = t_core // T_SLAB
    n_tc = T_SLAB // 128          # t-chunks per slab
    n_oc = D // OC                # 8 output chunks
    total = repeats * n_slab

    with tile.TileContext(nc) as tc:
        with tc.tile_pool(name="const", bufs=1) as cpool:
            xpool_cm = tc.tile_pool(name="xpool", bufs=XBUFS)
            xpool = xpool_cm.__enter__()
            xTpool_cm = tc.tile_pool(name="xT", bufs=2)
            xTpool = xTpool_cm.__enter__()
            opool_cm = tc.tile_pool(name="opool", bufs=3)
            opool = opool_cm.__enter__()
            spool_cm = tc.tile_pool(name="stage", bufs=1)
            spool = spool_cm.__enter__()

            ident = cpool.tile([128, 128], BF16)
            make_identity(nc, ident[:])

            # ---- x loads first (SWDGE cast f32->bf16, 2 MB per t-chunk) ----
            def load_x(s):
                t0 = (s % n_slab) * T_SLAB
                tiles = []
                for tcI in range(n_tc):
                    xt = xpool.tile([128, D], BF16, tag="xnat")
                    nc.gpsimd.dma_start(
                        xt[:], x[t0 + tcI * 128: t0 + (tcI + 1) * 128, :])
                    tiles.append(xt)
                return tiles

            xnat_q = {}
            xnat_q[0] = load_x(0)

            # params: cast-load bf16 via SWDGE
            v_sb = cpool.tile([128, NK, R], BF16)
            nc.gpsimd.dma_start(v_sb[:], V.rearrange("(a p) r -> p a r", p=128))

            alpha_row = spool.tile([1, 1], F32)
            nc.sync.dma_start(alpha_row[:], alpha[None, :])
            ones_t = spool.tile([1, 128], F32)
            nc.vector.memset(ones_t[:], 1.0)

            # ubT rows 0..63 = alpha * U^T (bf16); row 64 = raw bias
            ubT = cpool.tile([R + 1, NK, 128], BF16)
            nc.gpsimd.dma_start(ubT[R:R + 1, :, :], bias[None, :])

            u_stage = spool.tile([128, NK, R], BF16, tag="uv")
            nc.gpsimd.dma_start(u_stage[:], U.rearrange("(a p) r -> p a r", p=128))

            blk_view = blocks.rearrange("b (g p) i -> p (b g) i", p=128)

            if total > 1:
                xnat_q[1] = load_x(1)

            # ---- alpha broadcast to [128, 1] via transient PSUM pool ----
            alpha_col = cpool.tile([128, 1], F32)
            with tc.tile_pool(name="apsum", bufs=1, space="PSUM") as apool:
                a_ps = apool.tile([128, 512], F32, tag="aps")
                nc.tensor.matmul(a_ps[:, :1], ones_t[:], alpha_row[:],
                                 start=True, stop=True)
                nc.vector.tensor_copy(alpha_col[:], a_ps[:, :1])

            psum_cm = tc.tile_pool(name="psum", bufs=4, space="PSUM")
            psum = psum_cm.__enter__()
            tpsum_cm = tc.tile_pool(name="tpsum", bufs=3, space="PSUM")
            tpsum = tpsum_cm.__enter__()
            lrpsum_cm = tc.tile_pool(name="lrpsum", bufs=1, space="PSUM")
            lrpsum = lrpsum_cm.__enter__()

            blocksT = cpool.tile([128, NK, BI], BF16)

            # ---- steady-state phases ----
            xT_q = {}
            tlr_q = {}
            tlr_sb_q = {}

            def phaseA_start(i):
                xT_q[i] = xTpool.tile([128, NK, T_SLAB], BF16, tag="xT",
                                      name="xT_t")
                tlr_q[i] = lrpsum.tile([R, T_SLAB], F32, tag="tlr",
                                       name="tlr_t")

            def phaseA_group(i, oc, xnat_s):
                xT = xT_q[i]
                tlr = tlr_q[i]
                pt = tpsum.tile([128, 4, n_tc, 128], BF16, tag="tp")
                for kk in range(4):
                    ki = 4 * oc + kk
                    for tcI in range(n_tc):
                        nc.tensor.transpose(
                            pt[:, kk, tcI, :],
                            xnat_s[tcI][:, ki * 128:(ki + 1) * 128],
                            ident[:],
                        )
                nc.scalar.copy(xT[:, 4 * oc:4 * oc + 4, :], pt[:])
                for kk in range(4):
                    ki = 4 * oc + kk
                    nc.tensor.matmul(
                        tlr[:], v_sb[:, ki, :], xT[:, ki, :],
                        start=(ki == 0), stop=(ki == NK - 1),
                        skip_group_check=True,
                    )

            def phaseA_finish(i):
                tlr_sb = xTpool.tile([R + 1, T_SLAB], BF16, tag="tlr_sb")
                nc.vector.tensor_copy(tlr_sb[0:R, :], tlr_q[i][:])
                nc.vector.memset(tlr_sb[R:R + 1, :], 1.0)
                tlr_sb_q[i] = tlr_sb

            def phaseB_bd(i, oc):
                xT = xT_q[i]
                accs = []
                for tcI in range(n_tc):
                    acc = psum.tile([128, OC], F32, tag="acc")
                    for kk in range(4):
                        ki = 4 * oc + kk
                        nc.tensor.matmul(
                            acc[:, (kk // 2) * 256:(kk // 2) * 256 + 256],
                            xT[:, ki, tcI * 128:(tcI + 1) * 128],
                            blocksT[:, ki, :],
                            start=(kk == 0), stop=False,
                            skip_group_check=True,
                        )
                    accs.append(acc)
                return accs

            def phaseB_fin(i, oc, accs, o_sbs):
                tlr_sb = tlr_sb_q[i]
                for tcI in range(n_tc):
                    acc = accs[tcI]
                    nc.tensor.matmul(
                        acc[:], tlr_sb[:, tcI * 128:(tcI + 1) * 128],
                        ubT[:, 4 * oc:4 * oc + 4, :],
                        start=False, stop=True, skip_group_check=True,
                    )
                    nc.vector.tensor_copy(
                        o_sbs[tcI][:, oc * OC:(oc + 1) * OC], acc[:])

            # ---- slab 0 Phase A (PE busy while params finish staging) ----
            phaseA_start(0)
            for oc in range(n_oc):
                phaseA_group(0, oc, xnat_q[0])
            phaseA_finish(0)

            # ---- param transposes (bf16) + alpha-scaled copies ----
            def setup_blocks_round(rnd):
                blk_stage = spool.tile([128, NB, BI], BF16, tag="blk")
                nc.gpsimd.dma_start(blk_stage[:],
                                    blk_view[:, rnd * NB:(rnd + 1) * NB, :])
                for bb_ in range(NB // 2):
                    b = rnd * (NB // 2) + bb_
                    for ihalf in range(2):
                        ki = 2 * b + ihalf
                        pt = tpsum.tile([128, 1024], BF16, tag="tp")
                        for g in range(2):
                            nc.tensor.transpose(
                                pt[:, g * 128:(g + 1) * 128],
                                blk_stage[:, 2 * bb_ + g,
                                          ihalf * 128:(ihalf + 1) * 128],
                                ident[:],
                            )
                        nc.vector.tensor_scalar_mul(blocksT[:, ki, :],
                                                    pt[:, :256],
                                                    alpha_col[:, 0:1])

            setup_blocks_round(0)

            for j in range(NK // 4):
                up = tpsum.tile([128, 1024], BF16, tag="tp")
                for q in range(4):
                    a = 4 * j + q
                    nc.tensor.transpose(
                        up[:R, q * 128:(q + 1) * 128], u_stage[:, a, :],
                        ident[:])
                nc.vector.tensor_scalar_mul(ubT[0:R, 4 * j:4 * j + 4, :],
                                            up[:R, :512],
                                            alpha_col[:R, 0:1])

            setup_blocks_round(1)

            # ---- software-pipelined steady loop ----
            for it in range(total):
                s = it % n_slab
                nxt = it + 1
                if it + 2 < total:
                    xnat_q[it + 2] = load_x(it + 2)
                if nxt < total:
                    phaseA_start(nxt)
                o_sbs = [opool.tile([128, D], F32, tag="osb", name="o_sb")
                         for _ in range(n_tc)]
                pending = None
                for oc in range(n_oc):
                    accs = phaseB_bd(it, oc)
                    if nxt < total:
                        phaseA_group(nxt, oc, xnat_q[nxt])
                    if pending is not None:
                        phaseB_fin(it, pending[0], pending[1], o_sbs)
                    pending = (oc, accs)
                phaseB_fin(it, pending[0], pending[1], o_sbs)
                t0 = s * T_SLAB
                for tcI in range(n_tc):
                    nc.sync.dma_start(
                        out[t0 + tcI * 128: t0 + (tcI + 1) * 128, :],
                        o_sbs[tcI][:])
                if nxt < total:
                    phaseA_finish(nxt)
                # drop consumed refs
                xnat_q.pop(it, None)
                xT_q.pop(it, None)
                tlr_q.pop(it, None)
                tlr_sb_q.pop(it, None)

            spool_cm.__exit__(None, None, None)
            opool_cm.__exit__(None, None, None)
            xTpool_cm.__exit__(None, None, None)
            xpool_cm.__exit__(None, None, None)
            lrpsum_cm.__exit__(None, None, None)
            tpsum_cm.__exit__(None, None, None)
            psum_cm.__exit__(None, None, None)
    nc.compile()
    return nc


def check_waits(nc, verbose=True):
    bad = 0
    for fn in nc.m.functions:
        for bb in fn.blocks:
            for ins in bb.instructions:
                tname = type(ins).__name__
                if tname == "InstDrain":
                    continue
                nw = len(ins.sync_info.on_wait) if ins.sync_info else 0
                if tname == "InstEventSemaphore" and nw <= 2:
                    continue
                if nw > 1:
                    bad += 1
                    if verbose:
                        print("MULTI-WAIT", tname, ins.name,
                              [(w.ant_name, w.wait_value) for w in ins.sync_info.on_wait])
    return bad


_NC_CACHE = {}


def _get_nc(t_core, repeats=1):
    key = (t_core, repeats)
    if key not in _NC_CACHE:
        _NC_CACHE[key] = build(t_core, repeats)
    return _NC_CACHE[key]


def kernel(x, blocks, U, V, bias, alpha):
    batch_dims = x.shape[:-1]
    x_flat = np.ascontiguousarray(x.reshape(-1, D).astype(np.float32))
    n_tok = x_flat.shape[0]
    t_core = n_tok // N_CORES
    nc = _get_nc(t_core)

    blocks = np.ascontiguousarray(blocks, dtype=np.float32)
    U = np.ascontiguousarray(U, dtype=np.float32)
    V = np.ascontiguousarray(V, dtype=np.float32)
    bias = np.ascontiguousarray(bias, dtype=np.float32)
    alpha = np.ascontiguousarray(alpha, dtype=np.float32)

    in_maps = [
        {
            "x": x_flat[c * t_core:(c + 1) * t_core],
            "blocks": blocks, "U": U, "V": V, "bias": bias, "alpha": alpha,
        }
        for c in range(N_CORES)
    ]
    res = run_bass_kernel_spmd(nc, in_maps, list(range(N_CORES)))
    out = np.concatenate([res.results[c]["out"] for c in range(N_CORES)], axis=0)
    return out.reshape(*batch_dims, D)
